# revision 12
# baseline (speedup 1.0000x reference)
"""Trainium2 Bass kernel for nn_BiLinearDotLayer.

Computes, for feature (B,F,E)=(2048,200,64) f32 and weight (F,E,E):
    bilinear[b,i,d] = sum_e feature[b,i,e] * weight[i,e,d]
    out[b,i,j]      = sum_d bilinear[b,i,d] * feature[b,j,d]

Strategy (8 NeuronCores, data-parallel over batch):
  - Each core handles 256 batches packed as 128 (even, odd) pairs;
    partition = parity*64 + e so both parities use the full 128-wide
    DMA/PE width.  Weight replicated (both partition halves).
  - einsum1: per i, two quadrant matmuls (K=64,M=64,N=64) at PE tile
    positions (0,0)/(64,64); bilinear kept on-chip (fp16, batch-major)
  - einsum2: per pair, 4 matmuls: i-chunks {128, 72} x parity, N=200.
    i<128 lands on 128 PSUM partitions, i>=128 on 72 -> output DMAs
    span 128/72 partitions (not 100), engaging 16/12 SDMA engines.
  - Output written as fp16 (rel-err budget is 2e-2; fp16 adds ~3e-4),
    halving HBM write traffic; host converts to f32.
  - Schedule: load fpk/wpk in halves, e1(H0), then e1(H1) interleaved
    with e2(H0), then e2(H1); out-DMAs stream per 8-pair stage group.
"""

import os
import sys

for _p in ("/opt/trn_rl_repo", "/root/.axon_site/_ro/trn_rl_repo"):
    if os.path.isdir(_p) and _p not in sys.path:
        sys.path.insert(0, _p)

import numpy as np

B, F, E = 2048, 200, 64
NCORES = 8
BLOC = B // NCORES            # 256 batches per core
NPAIR = BLOC // 2             # 128 even/odd batch pairs per core
HP = NPAIR // 2               # pairs per half-shard
IGRP = 8                      # einsum1 i's per PSUM bank
SG = 8                        # pairs per staged out-DMA group
C1 = 128                      # einsum2 chunk1 rows (i 0:128)
C2 = F - C1                   # chunk2 rows (i 128:200) = 72

_RUNNER = None


def _build_program():
    import concourse.tile as tile
    from concourse import bacc, mybir

    f32 = mybir.dt.float32
    fp16 = mybir.dt.float16
    nc = bacc.Bacc("TRN2", target_bir_lowering=False, debug=False)

    # feature packed: fpk[par*64+e, bb*F+i] (fp16)
    fpk = nc.dram_tensor("fpk", [128, NPAIR * F], fp16, kind="ExternalInput")
    # weight packed: wpk[par*64+e, i*64+d] (fp16), both halves identical
    wpk = nc.dram_tensor("wpk", [128, F * E], fp16, kind="ExternalInput")
    # out[p, 0, bb, par, j] = out[2*bb+par, i=p, j]          (i < 128)
    # out[q, 1, bb, par, j] = out[2*bb+par, i=128+q, j]      (q < 72)
    out = nc.dram_tensor("out", [128, 2, NPAIR, 2, F], fp16, kind="ExternalOutput")
    outA_v = out.ap()[:, 0]
    outB_v = out.ap()[:, 1]

    with tile.TileContext(nc) as tc:
        with (
            tc.tile_pool(name="fpool", bufs=1) as fpool,
            tc.tile_pool(name="wpool", bufs=1) as wpool,
            tc.tile_pool(name="bpool", bufs=1) as bpool,
            tc.tile_pool(name="stA", bufs=3) as stApool,
            tc.tile_pool(name="stB", bufs=3) as stBpool,
        ):
            ftile = fpool.tile([128, NPAIR * F], fp16, name="ftile", tag="ftile")
            wtile = wpool.tile([128, F * E], fp16, name="wtile", tag="wtile")
            btile = bpool.tile([128, NPAIR * F], fp16, name="btile", tag="btile")

            # fpk first (both queues, quarters) so einsum1 can start ASAP;
            # wpk streams behind it in chunks (einsum1 groups dep per-chunk).
            fq = NPAIR * F // 4
            for q in range(4):
                eng = nc.sync if q % 2 == 0 else nc.scalar
                eng.dma_start(
                    out=ftile[:, q * fq : (q + 1) * fq],
                    in_=fpk.ap()[:, q * fq : (q + 1) * fq],
                )
            wq = F * E // 8
            for q in range(8):
                eng = nc.sync if q % 2 == 0 else nc.scalar
                eng.dma_start(
                    out=wtile[:, q * wq : (q + 1) * wq],
                    in_=wpk.ap()[:, q * wq : (q + 1) * wq],
                )

            f3 = ftile[:].rearrange("p (bb i) -> p bb i", i=F)
            b3 = btile[:].rearrange("p (bb i) -> p bb i", i=F)

            cpy = 0
            dma_i = 0

            def e1_group(ps1pool, i0):
                """einsum1 for i in [i0,i0+4) over all 128 pairs (N=128).

                Per i: two K=64,M=64,N=128 matmuls on disjoint PE quadrants
                (parity0 rows/cols 0:64, parity1 rows/cols 64:128); both
                write the same bank at disjoint partition ranges."""
                nonlocal cpy
                gs = min(4, F - i0)
                pst = ps1pool.tile([128, 512], f32, name="pst", tag="pst")
                for g in range(gs):
                    i = i0 + g
                    for par in (0, 1):
                        pr = slice(par * 64, par * 64 + 64)
                        nc.tensor.matmul(
                            out=pst[pr, g * NPAIR : (g + 1) * NPAIR],
                            lhsT=wtile[pr, i * E : (i + 1) * E],
                            rhs=f3[pr, :, i],
                            start=True,
                            stop=True,
                        )
                # cast fp32 psum -> fp16 bilinear, batch-major
                src = pst[:, : gs * NPAIR].rearrange("p (g bb) -> p bb g", bb=NPAIR)
                dst = b3[:, :, i0 : i0 + gs]
                if cpy % 2 == 0:
                    nc.vector.tensor_copy(out=dst, in_=src)
                else:
                    nc.scalar.copy(out=dst, in_=src)
                cpy += 1

            def e2_pair(psXpool, psYpool, bb, g, stageA, stageB):
                """einsum2 for pair bb -> stage slot g (fp16).

                bx/by are 2-bank psum tiles; each matmul owns one bank
                (parity0 at col 0, parity1 at col 512)."""
                bx = psXpool.tile([128, 1024], f32, name="bx", tag="bx")
                by = psYpool.tile([128, 1024], f32, name="by", tag="by")
                for par in (0, 1):
                    pr = slice(par * 64, par * 64 + 64)
                    nc.tensor.matmul(
                        out=bx[0:C1, par * 512 : par * 512 + F],
                        lhsT=b3[pr, bb, 0:C1],
                        rhs=f3[pr, bb, 0:F],
                        start=True,
                        stop=True,
                    )
                    nc.tensor.matmul(
                        out=by[0:C2, par * 512 : par * 512 + F],
                        lhsT=b3[pr, bb, C1:F],
                        rhs=f3[pr, bb, 0:F],
                        start=True,
                        stop=True,
                    )
                # evacuate psum -> fp16 stage (both parities in one op)
                sA = slice(g * 2 * F, (g + 1) * 2 * F)
                srcX = bx[:].rearrange("p (par k) -> p par k", par=2)[:C1, :, 0:F]
                srcY = by[:].rearrange("p (par k) -> p par k", par=2)[:C2, :, 0:F]
                dstA = stageA[:, sA].rearrange("p (par j) -> p par j", par=2)
                dstB = stageB[0:C2, sA].rearrange("p (par j) -> p par j", par=2)
                nc.vector.tensor_copy(out=dstA, in_=srcX)
                nc.scalar.copy(out=dstB, in_=srcY)

            def stage_tiles():
                stageA = stApool.tile(
                    [128, SG * 2 * F], fp16, name="stageA", tag="stageA"
                )
                stageB = stBpool.tile(
                    [128, SG * 2 * F], fp16, name="stageB", tag="stageB"
                )
                return stageA, stageB

            def stage_flush(bb0, stageA, stageB):
                nonlocal dma_i
                eA = nc.sync if dma_i % 2 == 0 else nc.scalar
                eB = nc.scalar if dma_i % 2 == 0 else nc.sync
                dma_i += 1
                eA.dma_start(
                    out=outA_v[:, bb0 : bb0 + SG, :, :],
                    in_=stageA[0:C1, :].rearrange(
                        "p (b par j) -> p b par j", par=2, j=F
                    ),
                )
                eB.dma_start(
                    out=outB_v[0:C2, bb0 : bb0 + SG, :, :],
                    in_=stageB[0:C2, :].rearrange(
                        "p (b par j) -> p b par j", par=2, j=F
                    ),
                )

            # ---- schedule ----
            with tc.tile_pool(name="ps1", bufs=2, space="PSUM") as ps1pool:
                for i0 in range(0, F, 4):
                    e1_group(ps1pool, i0)

            stageA = stageB = None
            with (
                tc.tile_pool(name="psX", bufs=2, space="PSUM") as psXpool,
                tc.tile_pool(name="psY", bufs=2, space="PSUM") as psYpool,
            ):
                for bb in range(NPAIR):
                    if bb % SG == 0:
                        stageA, stageB = stage_tiles()
                    e2_pair(psXpool, psYpool, bb, bb % SG, stageA, stageB)
                    if bb % SG == SG - 1:
                        stage_flush(bb - (SG - 1), stageA, stageB)

    nc.compile()
    return nc


class _Runner:
    """Builds the program once and keeps a reusable sharded jit."""

    def __init__(self):
        self.nc = _build_program()
        import jax
        from jax.sharding import Mesh, PartitionSpec
        from jax.experimental.shard_map import shard_map
        from concourse import mybir
        from concourse import bass2jax

        bass2jax.install_neuronx_cc_hook()
        nc = self.nc

        partition_name = (
            nc.partition_id_tensor.name if nc.partition_id_tensor else None
        )
        in_names, out_names, out_avals, zero_outs = [], [], [], []
        for alloc in nc.m.functions[0].allocations:
            if not isinstance(alloc, mybir.MemoryLocationSet):
                continue
            name = alloc.memorylocations[0].name
            if alloc.kind == "ExternalInput":
                if name != partition_name:
                    in_names.append(name)
            elif alloc.kind == "ExternalOutput":
                shape = tuple(alloc.tensor_shape)
                dtype = mybir.dt.np(alloc.dtype)
                out_names.append(name)
                out_avals.append(jax.core.ShapedArray(shape, dtype))
                zero_outs.append(np.zeros(shape, dtype))
        self.in_names = list(in_names)
        self.out_names = out_names
        self.out_avals = out_avals
        self.zero_outs = zero_outs
        n_params = len(in_names)
        n_outs = len(out_avals)
        in_names_full = in_names + out_names
        if partition_name is not None:
            in_names_full.append(partition_name)
        donate = tuple(range(n_params, n_params + n_outs))

        def _body(*args):
            operands = list(args)
            if partition_name is not None:
                operands.append(bass2jax.partition_id_tensor())
            outs = bass2jax._bass_exec_p.bind(
                *operands,
                out_avals=tuple(out_avals),
                in_names=tuple(in_names_full),
                out_names=tuple(out_names),
                lowering_input_output_aliases=(),
                sim_require_finite=True,
                sim_require_nnan=True,
                nc=nc,
            )
            return tuple(outs)

        devices = jax.devices()[:NCORES]
        mesh = Mesh(np.asarray(devices), ("core",))
        in_specs = (PartitionSpec("core"),) * (n_params + n_outs)
        out_specs = (PartitionSpec("core"),) * n_outs
        self.sharded = jax.jit(
            shard_map(
                _body,
                mesh=mesh,
                in_specs=in_specs,
                out_specs=out_specs,
                check_rep=False,
            ),
            donate_argnums=donate,
            keep_unused=True,
        )

    def run(self, concat_inputs):
        """concat_inputs: dict name -> (8*shape0, ...) array."""
        args = [concat_inputs[n] for n in self.in_names]
        zeros = [
            np.zeros((NCORES * z.shape[0], *z.shape[1:]), z.dtype)
            for z in self.zero_outs
        ]
        outs = self.sharded(*args, *zeros)
        return {n: np.asarray(outs[i]) for i, n in enumerate(self.out_names)}


def _get_runner():
    global _RUNNER
    if _RUNNER is None:
        _RUNNER = _Runner()
    return _RUNNER


def pack_inputs(feature, weight):
    """Host-side packing: returns dict of concatenated per-core inputs."""
    feature = np.ascontiguousarray(np.asarray(feature, dtype=np.float32))
    weight = np.ascontiguousarray(np.asarray(weight, dtype=np.float32))
    # fpk[core][par*64+e, bb*F+i] = feature[core*BLOC + 2*bb + par, i, e]
    ft = feature.reshape(NCORES, NPAIR, 2, F, E)  # [core, bb, par, i, e]
    fpk = (
        np.ascontiguousarray(ft.transpose(0, 2, 4, 1, 3))
        .reshape(NCORES * 128, NPAIR * F)
        .astype(np.float16)
    )
    wt = np.ascontiguousarray(weight.transpose(1, 0, 2)).reshape(E, F * E)
    wpk_one = np.concatenate([wt, wt], axis=0).astype(np.float16)
    wpk = np.tile(wpk_one, (NCORES, 1))
    return {"fpk": fpk, "wpk": wpk}


def unpack_output(out_dev):
    """out_dev: (8*128, 2, NPAIR, 2, F) device layout -> (B, F, F)."""
    o = out_dev.reshape(NCORES, 128, 2, NPAIR, 2, F)
    oA = o[:, :, 0].transpose(0, 2, 3, 1, 4)          # [c, bb, par, i(128), j]
    oB = o[:, :C2, 1].transpose(0, 2, 3, 1, 4)        # [c, bb, par, q(72), j]
    out = np.concatenate(
        [oA.reshape(NCORES, BLOC, C1, F), oB.reshape(NCORES, BLOC, C2, F)],
        axis=2,
    )
    return np.ascontiguousarray(out).reshape(B, F, F).astype(np.float32)


def kernel(feature, weight):
    r = _get_runner()
    ins = pack_inputs(feature, weight)
    outs = r.run(ins)
    return unpack_output(outs["out"])


if __name__ == "__main__":
    rng = np.random.default_rng(0)
    feature = rng.standard_normal((B, F, E), dtype=np.float32)
    weight = (0.01 * rng.standard_normal((F, E, E))).astype(np.float32)
    got = kernel(feature, weight)
    bil = np.einsum(
        "bie,ied->bid", feature.astype(np.float64), weight.astype(np.float64)
    )
    ref = np.einsum("bid,bjd->bij", bil, feature.astype(np.float64))
    err = np.abs(got - ref)
    denom = np.abs(ref).max()
    print("max abs err:", err.max(), "rel(scale):", err.max() / denom)
    l2 = np.linalg.norm((got - ref).ravel()) / np.linalg.norm(ref.ravel())
    print("L2 rel:", l2)


# revision 13
# speedup vs baseline: 1.0174x; 1.0174x over previous
"""Trainium2 Bass kernel for nn_BiLinearDotLayer.

Computes, for feature (B,F,E)=(2048,200,64) f32 and weight (F,E,E):
    bilinear[b,i,d] = sum_e feature[b,i,e] * weight[i,e,d]
    out[b,i,j]      = sum_d bilinear[b,i,d] * feature[b,j,d]

Strategy (8 NeuronCores, data-parallel over batch):
  - Each core handles 256 batches packed as 128 (even, odd) pairs;
    partition = parity*64 + e so both parities use the full 128-wide
    DMA/PE width.  Weight replicated (both partition halves).
  - einsum1: per i, two quadrant matmuls (K=64,M=64,N=64) at PE tile
    positions (0,0)/(64,64); bilinear kept on-chip (fp16, batch-major)
  - einsum2: per pair, 4 matmuls: i-chunks {128, 72} x parity, N=200.
    i<128 lands on 128 PSUM partitions, i>=128 on 72 -> output DMAs
    span 128/72 partitions (not 100), engaging 16/12 SDMA engines.
  - Output written as fp16 (rel-err budget is 2e-2; fp16 adds ~3e-4),
    halving HBM write traffic; host converts to f32.
  - Schedule: load fpk/wpk in halves, e1(H0), then e1(H1) interleaved
    with e2(H0), then e2(H1); out-DMAs stream per 8-pair stage group.
"""

import os
import sys

for _p in ("/opt/trn_rl_repo", "/root/.axon_site/_ro/trn_rl_repo"):
    if os.path.isdir(_p) and _p not in sys.path:
        sys.path.insert(0, _p)

import numpy as np

B, F, E = 2048, 200, 64
NCORES = 8
BLOC = B // NCORES            # 256 batches per core
NPAIR = BLOC // 2             # 128 even/odd batch pairs per core
HP = NPAIR // 2               # pairs per half-shard
IGRP = 8                      # einsum1 i's per PSUM bank
SG = 8                        # pairs per staged out-DMA group
C1 = 128                      # einsum2 chunk1 rows (i 0:128)
C2 = F - C1                   # chunk2 rows (i 128:200) = 72

_RUNNER = None


def _build_program():
    import concourse.tile as tile
    from concourse import bacc, mybir

    f32 = mybir.dt.float32
    fp16 = mybir.dt.float16
    nc = bacc.Bacc("TRN2", target_bir_lowering=False, debug=False)

    # feature packed: fpk[par*64+e, bb*F+i] (fp16)
    fpk = nc.dram_tensor("fpk", [128, NPAIR * F], fp16, kind="ExternalInput")
    # weight packed block-diagonal: wpk[e+64h, i*128 + h*64 + d] (fp16)
    wpk = nc.dram_tensor("wpk", [128, F * 2 * E], fp16, kind="ExternalInput")
    # out[p, 0, bb, par, j] = out[2*bb+par, i=p, j]          (i < 128)
    # out[q, 1, bb, par, j] = out[2*bb+par, i=128+q, j]      (q < 72)
    out = nc.dram_tensor("out", [128, 2, NPAIR, 2, F], fp16, kind="ExternalOutput")
    outA_v = out.ap()[:, 0]
    outB_v = out.ap()[:, 1]

    with tile.TileContext(nc) as tc:
        with (
            tc.tile_pool(name="fpool", bufs=1) as fpool,
            tc.tile_pool(name="wpool", bufs=1) as wpool,
            tc.tile_pool(name="bpool", bufs=1) as bpool,
            tc.tile_pool(name="stA", bufs=3) as stApool,
            tc.tile_pool(name="stB", bufs=3) as stBpool,
        ):
            ftile = fpool.tile([128, NPAIR * F], fp16, name="ftile", tag="ftile")
            wtile = wpool.tile([128, F * 2 * E], fp16, name="wtile", tag="wtile")
            btile = bpool.tile([128, NPAIR * F], fp16, name="btile", tag="btile")

            # fpk first (both queues, quarters) so einsum1 can start ASAP;
            # wpk streams behind it in chunks (einsum1 groups dep per-chunk).
            fq = NPAIR * F // 4
            for q in range(4):
                eng = nc.sync if q % 2 == 0 else nc.scalar
                eng.dma_start(
                    out=ftile[:, q * fq : (q + 1) * fq],
                    in_=fpk.ap()[:, q * fq : (q + 1) * fq],
                )
            wq = F * 2 * E // 8
            for q in range(8):
                eng = nc.sync if q % 2 == 0 else nc.scalar
                eng.dma_start(
                    out=wtile[:, q * wq : (q + 1) * wq],
                    in_=wpk.ap()[:, q * wq : (q + 1) * wq],
                )

            f3 = ftile[:].rearrange("p (bb i) -> p bb i", i=F)
            b3 = btile[:].rearrange("p (bb i) -> p bb i", i=F)

            cpy = 0
            dma_i = 0

            def e1_group(ps1pool, i0):
                """einsum1 for i in [i0,i0+4) over all 128 pairs (N=128).

                Per i: two K=64,M=64,N=128 matmuls on disjoint PE quadrants
                (parity0 rows/cols 0:64, parity1 rows/cols 64:128); both
                write the same bank at disjoint partition ranges."""
                nonlocal cpy
                gs = min(4, F - i0)
                pst = ps1pool.tile([128, 512], f32, name="pst", tag="pst")
                for g in range(gs):
                    i = i0 + g
                    nc.tensor.matmul(
                        out=pst[:, g * NPAIR : (g + 1) * NPAIR],
                        lhsT=wtile[:, i * 2 * E : (i + 1) * 2 * E],
                        rhs=f3[:, :, i],
                        start=True,
                        stop=True,
                    )
                # cast fp32 psum -> fp16 bilinear, batch-major
                src = pst[:, : gs * NPAIR].rearrange("p (g bb) -> p bb g", bb=NPAIR)
                dst = b3[:, :, i0 : i0 + gs]
                if cpy % 2 == 0:
                    nc.vector.tensor_copy(out=dst, in_=src)
                else:
                    nc.scalar.copy(out=dst, in_=src)
                cpy += 1

            def e2_pair(psXpool, psYpool, bb, g, stageA, stageB):
                """einsum2 for pair bb -> stage slot g (fp16).

                bx/by are 2-bank psum tiles; each matmul owns one bank
                (parity0 at col 0, parity1 at col 512)."""
                bx = psXpool.tile([128, 1024], f32, name="bx", tag="bx")
                by = psYpool.tile([128, 1024], f32, name="by", tag="by")
                for par in (0, 1):
                    pr = slice(par * 64, par * 64 + 64)
                    nc.tensor.matmul(
                        out=bx[0:C1, par * 512 : par * 512 + F],
                        lhsT=b3[pr, bb, 0:C1],
                        rhs=f3[pr, bb, 0:F],
                        start=True,
                        stop=True,
                    )
                    nc.tensor.matmul(
                        out=by[0:C2, par * 512 : par * 512 + F],
                        lhsT=b3[pr, bb, C1:F],
                        rhs=f3[pr, bb, 0:F],
                        start=True,
                        stop=True,
                    )
                # evacuate psum -> fp16 stage (both parities in one op)
                sA = slice(g * 2 * F, (g + 1) * 2 * F)
                srcX = bx[:].rearrange("p (par k) -> p par k", par=2)[:C1, :, 0:F]
                srcY = by[:].rearrange("p (par k) -> p par k", par=2)[:C2, :, 0:F]
                dstA = stageA[:, sA].rearrange("p (par j) -> p par j", par=2)
                dstB = stageB[0:C2, sA].rearrange("p (par j) -> p par j", par=2)
                nc.vector.tensor_copy(out=dstA, in_=srcX)
                nc.scalar.copy(out=dstB, in_=srcY)

            def stage_tiles():
                stageA = stApool.tile(
                    [128, SG * 2 * F], fp16, name="stageA", tag="stageA"
                )
                stageB = stBpool.tile(
                    [128, SG * 2 * F], fp16, name="stageB", tag="stageB"
                )
                return stageA, stageB

            def stage_flush(bb0, stageA, stageB):
                nonlocal dma_i
                eA = nc.sync if dma_i % 2 == 0 else nc.scalar
                eB = nc.scalar if dma_i % 2 == 0 else nc.sync
                dma_i += 1
                eA.dma_start(
                    out=outA_v[:, bb0 : bb0 + SG, :, :],
                    in_=stageA[0:C1, :].rearrange(
                        "p (b par j) -> p b par j", par=2, j=F
                    ),
                )
                eB.dma_start(
                    out=outB_v[0:C2, bb0 : bb0 + SG, :, :],
                    in_=stageB[0:C2, :].rearrange(
                        "p (b par j) -> p b par j", par=2, j=F
                    ),
                )

            # ---- schedule ----
            with tc.tile_pool(name="ps1", bufs=2, space="PSUM") as ps1pool:
                for i0 in range(0, F, 4):
                    e1_group(ps1pool, i0)

            stageA = stageB = None
            with (
                tc.tile_pool(name="psX", bufs=2, space="PSUM") as psXpool,
                tc.tile_pool(name="psY", bufs=2, space="PSUM") as psYpool,
            ):
                for bb in range(NPAIR):
                    if bb % SG == 0:
                        stageA, stageB = stage_tiles()
                    e2_pair(psXpool, psYpool, bb, bb % SG, stageA, stageB)
                    if bb % SG == SG - 1:
                        stage_flush(bb - (SG - 1), stageA, stageB)

    nc.compile()
    return nc


class _Runner:
    """Builds the program once and keeps a reusable sharded jit."""

    def __init__(self):
        self.nc = _build_program()
        import jax
        from jax.sharding import Mesh, PartitionSpec
        from jax.experimental.shard_map import shard_map
        from concourse import mybir
        from concourse import bass2jax

        bass2jax.install_neuronx_cc_hook()
        nc = self.nc

        partition_name = (
            nc.partition_id_tensor.name if nc.partition_id_tensor else None
        )
        in_names, out_names, out_avals, zero_outs = [], [], [], []
        for alloc in nc.m.functions[0].allocations:
            if not isinstance(alloc, mybir.MemoryLocationSet):
                continue
            name = alloc.memorylocations[0].name
            if alloc.kind == "ExternalInput":
                if name != partition_name:
                    in_names.append(name)
            elif alloc.kind == "ExternalOutput":
                shape = tuple(alloc.tensor_shape)
                dtype = mybir.dt.np(alloc.dtype)
                out_names.append(name)
                out_avals.append(jax.core.ShapedArray(shape, dtype))
                zero_outs.append(np.zeros(shape, dtype))
        self.in_names = list(in_names)
        self.out_names = out_names
        self.out_avals = out_avals
        self.zero_outs = zero_outs
        n_params = len(in_names)
        n_outs = len(out_avals)
        in_names_full = in_names + out_names
        if partition_name is not None:
            in_names_full.append(partition_name)
        donate = tuple(range(n_params, n_params + n_outs))

        def _body(*args):
            operands = list(args)
            if partition_name is not None:
                operands.append(bass2jax.partition_id_tensor())
            outs = bass2jax._bass_exec_p.bind(
                *operands,
                out_avals=tuple(out_avals),
                in_names=tuple(in_names_full),
                out_names=tuple(out_names),
                lowering_input_output_aliases=(),
                sim_require_finite=True,
                sim_require_nnan=True,
                nc=nc,
            )
            return tuple(outs)

        devices = jax.devices()[:NCORES]
        mesh = Mesh(np.asarray(devices), ("core",))
        in_specs = (PartitionSpec("core"),) * (n_params + n_outs)
        out_specs = (PartitionSpec("core"),) * n_outs
        self.sharded = jax.jit(
            shard_map(
                _body,
                mesh=mesh,
                in_specs=in_specs,
                out_specs=out_specs,
                check_rep=False,
            ),
            donate_argnums=donate,
            keep_unused=True,
        )

    def run(self, concat_inputs):
        """concat_inputs: dict name -> (8*shape0, ...) array."""
        args = [concat_inputs[n] for n in self.in_names]
        zeros = [
            np.zeros((NCORES * z.shape[0], *z.shape[1:]), z.dtype)
            for z in self.zero_outs
        ]
        outs = self.sharded(*args, *zeros)
        return {n: np.asarray(outs[i]) for i, n in enumerate(self.out_names)}


def _get_runner():
    global _RUNNER
    if _RUNNER is None:
        _RUNNER = _Runner()
    return _RUNNER


def pack_inputs(feature, weight):
    """Host-side packing: returns dict of concatenated per-core inputs."""
    feature = np.ascontiguousarray(np.asarray(feature, dtype=np.float32))
    weight = np.ascontiguousarray(np.asarray(weight, dtype=np.float32))
    # fpk[core][par*64+e, bb*F+i] = feature[core*BLOC + 2*bb + par, i, e]
    ft = feature.reshape(NCORES, NPAIR, 2, F, E)  # [core, bb, par, i, e]
    fpk = (
        np.ascontiguousarray(ft.transpose(0, 2, 4, 1, 3))
        .reshape(NCORES * 128, NPAIR * F)
        .astype(np.float16)
    )
    # block-diagonal: wpk_one[e+64h, i*128 + h*64 + d] = w[i,e,d]
    wbd = np.zeros((2, E, F, 2, E), np.float16)  # [h, e, i, h', d]
    wt = weight.transpose(1, 0, 2).astype(np.float16)  # [e, i, d]
    wbd[0, :, :, 0, :] = wt
    wbd[1, :, :, 1, :] = wt
    wpk_one = wbd.reshape(128, F * 2 * E)
    wpk = np.tile(wpk_one, (NCORES, 1))
    return {"fpk": fpk, "wpk": wpk}


def unpack_output(out_dev):
    """out_dev: (8*128, 2, NPAIR, 2, F) device layout -> (B, F, F)."""
    o = out_dev.reshape(NCORES, 128, 2, NPAIR, 2, F)
    oA = o[:, :, 0].transpose(0, 2, 3, 1, 4)          # [c, bb, par, i(128), j]
    oB = o[:, :C2, 1].transpose(0, 2, 3, 1, 4)        # [c, bb, par, q(72), j]
    out = np.concatenate(
        [oA.reshape(NCORES, BLOC, C1, F), oB.reshape(NCORES, BLOC, C2, F)],
        axis=2,
    )
    return np.ascontiguousarray(out).reshape(B, F, F).astype(np.float32)


def kernel(feature, weight):
    r = _get_runner()
    ins = pack_inputs(feature, weight)
    outs = r.run(ins)
    return unpack_output(outs["out"])


if __name__ == "__main__":
    rng = np.random.default_rng(0)
    feature = rng.standard_normal((B, F, E), dtype=np.float32)
    weight = (0.01 * rng.standard_normal((F, E, E))).astype(np.float32)
    got = kernel(feature, weight)
    bil = np.einsum(
        "bie,ied->bid", feature.astype(np.float64), weight.astype(np.float64)
    )
    ref = np.einsum("bid,bjd->bij", bil, feature.astype(np.float64))
    err = np.abs(got - ref)
    denom = np.abs(ref).max()
    print("max abs err:", err.max(), "rel(scale):", err.max() / denom)
    l2 = np.linalg.norm((got - ref).ravel()) / np.linalg.norm(ref.ravel())
    print("L2 rel:", l2)


# revision 16
# speedup vs baseline: 1.1466x; 1.1270x over previous
"""Trainium2 Bass kernel for nn_BiLinearDotLayer.

Computes, for feature (B,F,E)=(2048,200,64) f32 and weight (F,E,E):
    bilinear[b,i,d] = sum_e feature[b,i,e] * weight[i,e,d]
    out[b,i,j]      = sum_d bilinear[b,i,d] * feature[b,j,d]

Strategy (8 NeuronCores, data-parallel over batch):
  - Each core handles 256 batches packed as 128 (even, odd) pairs;
    partition = parity*64 + e so both parities use the full 128-wide
    DMA/PE width.  Weight replicated (both partition halves).
  - einsum1: per i, ONE K=128,M=128,N=128 matmul using a block-diagonal
    weight layout [[w_i,0],[0,w_i]] packed on host (both parities in one
    pass, half the PE instructions); bilinear on-chip (fp16, batch-major)
  - einsum2: per pair, 4 matmuls: i-chunks {128, 72} x parity, N=200.
    i<128 lands on 128 PSUM partitions, i>=128 on 72 -> output DMAs
    span 128/72 partitions (not 100), engaging 16/12 SDMA engines.
  - Output written as fp16 (rel-err budget is 2e-2; fp16 adds ~3e-4),
    halving HBM write traffic; host converts to f32.
  - Schedule: load fpk/wpk in halves, e1(H0), then e1(H1) interleaved
    with e2(H0), then e2(H1); out-DMAs stream per 8-pair stage group.
"""

import os
import sys

for _p in ("/opt/trn_rl_repo", "/root/.axon_site/_ro/trn_rl_repo"):
    if os.path.isdir(_p) and _p not in sys.path:
        sys.path.insert(0, _p)

import numpy as np

B, F, E = 2048, 200, 64
NCORES = 8
BLOC = B // NCORES            # 256 batches per core
NPAIR = BLOC // 2             # 128 even/odd batch pairs per core
HP = NPAIR // 2               # pairs per half-shard
IGRP = 8                      # einsum1 i's per PSUM bank
SG = 8                        # pairs per staged out-DMA group
C1 = 128                      # einsum2 chunk1 rows (i 0:128)
C2 = F - C1                   # chunk2 rows (i 128:200) = 72

_RUNNER = None


def _build_program():
    import concourse.tile as tile
    from concourse import bacc, mybir

    f32 = mybir.dt.float32
    fp16 = mybir.dt.float16
    nc = bacc.Bacc("TRN2", target_bir_lowering=False, debug=False)

    # feature packed: fpk[par*64+e, bb*F+i] (fp16)
    fpk = nc.dram_tensor("fpk", [128, NPAIR * F], fp16, kind="ExternalInput")
    # weight packed block-diagonal: wpk[e+64h, i*128 + h*64 + d] (fp16)
    wpk = nc.dram_tensor("wpk", [128, F * 2 * E], fp16, kind="ExternalInput")
    # out[p, 0, bb, par, j] = out[2*bb+par, i=p, j]          (i < 128)
    # out[q, 1, bb, par, j] = out[2*bb+par, i=128+q, j]      (q < 72)
    out = nc.dram_tensor("out", [128, 2, NPAIR, 2, F], fp16, kind="ExternalOutput")
    outA_v = out.ap()[:, 0]
    outB_v = out.ap()[:, 1]

    with tile.TileContext(nc) as tc:
        with (
            tc.tile_pool(name="fpool", bufs=1) as fpool,
            tc.tile_pool(name="wpool", bufs=1) as wpool,
            tc.tile_pool(name="bpool", bufs=1) as bpool,
            tc.tile_pool(name="stA", bufs=4) as stApool,
            tc.tile_pool(name="stB", bufs=4) as stBpool,
        ):
            ftile = fpool.tile([128, NPAIR * F], fp16, name="ftile", tag="ftile")
            wtile = wpool.tile([128, F * 2 * E], fp16, name="wtile", tag="wtile")
            btile = bpool.tile([128, NPAIR * F], fp16, name="btile", tag="btile")

            # fpk first (both queues, quarters) so einsum1 can start ASAP;
            # wpk streams behind it in chunks (einsum1 groups dep per-chunk).
            fq = NPAIR * F // 4
            for q in range(4):
                eng = nc.sync if q % 2 == 0 else nc.scalar
                eng.dma_start(
                    out=ftile[:, q * fq : (q + 1) * fq],
                    in_=fpk.ap()[:, q * fq : (q + 1) * fq],
                )
            wq = F * 2 * E // 8
            for q in range(8):
                eng = nc.sync if q % 2 == 0 else nc.scalar
                eng.dma_start(
                    out=wtile[:, q * wq : (q + 1) * wq],
                    in_=wpk.ap()[:, q * wq : (q + 1) * wq],
                )

            f3 = ftile[:].rearrange("p (bb i) -> p bb i", i=F)
            b3 = btile[:].rearrange("p (bb i) -> p bb i", i=F)

            cpy = 0
            dma_i = 0

            def e1_group(ps1pool, i0):
                """einsum1 for i in [i0,i0+4) over all 128 pairs (N=128).

                Per i: two K=64,M=64,N=128 matmuls on disjoint PE quadrants
                (parity0 rows/cols 0:64, parity1 rows/cols 64:128); both
                write the same bank at disjoint partition ranges."""
                nonlocal cpy
                gs = min(4, F - i0)
                pst = ps1pool.tile([128, 512], f32, name="pst", tag="pst")
                for g in range(gs):
                    i = i0 + g
                    nc.tensor.matmul(
                        out=pst[:, g * NPAIR : (g + 1) * NPAIR],
                        lhsT=wtile[:, i * 2 * E : (i + 1) * 2 * E],
                        rhs=f3[:, :, i],
                        start=True,
                        stop=True,
                    )
                # cast fp32 psum -> fp16 bilinear, batch-major
                src = pst[:, : gs * NPAIR].rearrange("p (g bb) -> p bb g", bb=NPAIR)
                dst = b3[:, :, i0 : i0 + gs]
                if cpy % 2 == 0:
                    nc.vector.tensor_copy(out=dst, in_=src)
                else:
                    nc.scalar.copy(out=dst, in_=src)
                cpy += 1

            def e2_pair(psXpool, psYpool, bb, g, stageA, stageB):
                """einsum2 for pair bb -> stage slot g (fp16).

                bx/by are 2-bank psum tiles; each matmul owns one bank
                (parity0 at col 0, parity1 at col 512)."""
                bx = psXpool.tile([128, 1024], f32, name="bx", tag="bx")
                by = psYpool.tile([128, 1024], f32, name="by", tag="by")
                for par in (0, 1):
                    pr = slice(par * 64, par * 64 + 64)
                    nc.tensor.matmul(
                        out=bx[0:C1, par * 512 : par * 512 + F],
                        lhsT=b3[pr, bb, 0:C1],
                        rhs=f3[pr, bb, 0:F],
                        start=True,
                        stop=True,
                    )
                    nc.tensor.matmul(
                        out=by[0:C2, par * 512 : par * 512 + F],
                        lhsT=b3[pr, bb, C1:F],
                        rhs=f3[pr, bb, 0:F],
                        start=True,
                        stop=True,
                    )
                # evacuate psum -> fp16 stage (both parities in one op)
                sA = slice(g * 2 * F, (g + 1) * 2 * F)
                srcX = bx[:].rearrange("p (par k) -> p par k", par=2)[:C1, :, 0:F]
                srcY = by[:].rearrange("p (par k) -> p par k", par=2)[:C2, :, 0:F]
                dstA = stageA[:, sA].rearrange("p (par j) -> p par j", par=2)
                dstB = stageB[0:C2, sA].rearrange("p (par j) -> p par j", par=2)
                nc.vector.tensor_copy(out=dstA, in_=srcX)
                nc.scalar.copy(out=dstB, in_=srcY)

            def stage_tiles():
                stageA = stApool.tile(
                    [128, SG * 2 * F], fp16, name="stageA", tag="stageA"
                )
                stageB = stBpool.tile(
                    [128, SG * 2 * F], fp16, name="stageB", tag="stageB"
                )
                return stageA, stageB

            def stage_flush(bb0, stageA, stageB):
                nonlocal dma_i
                eA = nc.sync
                eB = nc.sync
                dma_i += 1
                eA.dma_start(
                    out=outA_v[:, bb0 : bb0 + SG, :, :],
                    in_=stageA[0:C1, :].rearrange(
                        "p (b par j) -> p b par j", par=2, j=F
                    ),
                )
                eB.dma_start(
                    out=outB_v[0:C2, bb0 : bb0 + SG, :, :],
                    in_=stageB[0:C2, :].rearrange(
                        "p (b par j) -> p b par j", par=2, j=F
                    ),
                )

            # ---- schedule ----
            with tc.tile_pool(name="ps1", bufs=2, space="PSUM") as ps1pool:
                for i0 in range(0, F, 4):
                    e1_group(ps1pool, i0)

            stageA = stageB = None
            with (
                tc.tile_pool(name="psX", bufs=2, space="PSUM") as psXpool,
                tc.tile_pool(name="psY", bufs=2, space="PSUM") as psYpool,
            ):
                for bb in range(NPAIR):
                    if bb % SG == 0:
                        stageA, stageB = stage_tiles()
                    e2_pair(psXpool, psYpool, bb, bb % SG, stageA, stageB)
                    if bb % SG == SG - 1:
                        stage_flush(bb - (SG - 1), stageA, stageB)

    nc.compile()
    return nc


class _Runner:
    """Builds the program once and keeps a reusable sharded jit."""

    def __init__(self):
        self.nc = _build_program()
        import jax
        from jax.sharding import Mesh, PartitionSpec
        from jax.experimental.shard_map import shard_map
        from concourse import mybir
        from concourse import bass2jax

        bass2jax.install_neuronx_cc_hook()
        nc = self.nc

        partition_name = (
            nc.partition_id_tensor.name if nc.partition_id_tensor else None
        )
        in_names, out_names, out_avals, zero_outs = [], [], [], []
        for alloc in nc.m.functions[0].allocations:
            if not isinstance(alloc, mybir.MemoryLocationSet):
                continue
            name = alloc.memorylocations[0].name
            if alloc.kind == "ExternalInput":
                if name != partition_name:
                    in_names.append(name)
            elif alloc.kind == "ExternalOutput":
                shape = tuple(alloc.tensor_shape)
                dtype = mybir.dt.np(alloc.dtype)
                out_names.append(name)
                out_avals.append(jax.core.ShapedArray(shape, dtype))
                zero_outs.append(np.zeros(shape, dtype))
        self.in_names = list(in_names)
        self.out_names = out_names
        self.out_avals = out_avals
        self.zero_outs = zero_outs
        n_params = len(in_names)
        n_outs = len(out_avals)
        in_names_full = in_names + out_names
        if partition_name is not None:
            in_names_full.append(partition_name)
        donate = tuple(range(n_params, n_params + n_outs))

        def _body(*args):
            operands = list(args)
            if partition_name is not None:
                operands.append(bass2jax.partition_id_tensor())
            outs = bass2jax._bass_exec_p.bind(
                *operands,
                out_avals=tuple(out_avals),
                in_names=tuple(in_names_full),
                out_names=tuple(out_names),
                lowering_input_output_aliases=(),
                sim_require_finite=True,
                sim_require_nnan=True,
                nc=nc,
            )
            return tuple(outs)

        devices = jax.devices()[:NCORES]
        mesh = Mesh(np.asarray(devices), ("core",))
        in_specs = (PartitionSpec("core"),) * (n_params + n_outs)
        out_specs = (PartitionSpec("core"),) * n_outs
        self.sharded = jax.jit(
            shard_map(
                _body,
                mesh=mesh,
                in_specs=in_specs,
                out_specs=out_specs,
                check_rep=False,
            ),
            donate_argnums=donate,
            keep_unused=True,
        )

    def run(self, concat_inputs):
        """concat_inputs: dict name -> (8*shape0, ...) array."""
        args = [concat_inputs[n] for n in self.in_names]
        zeros = [
            np.zeros((NCORES * z.shape[0], *z.shape[1:]), z.dtype)
            for z in self.zero_outs
        ]
        outs = self.sharded(*args, *zeros)
        return {n: np.asarray(outs[i]) for i, n in enumerate(self.out_names)}


def _get_runner():
    global _RUNNER
    if _RUNNER is None:
        _RUNNER = _Runner()
    return _RUNNER


def pack_inputs(feature, weight):
    """Host-side packing: returns dict of concatenated per-core inputs."""
    feature = np.ascontiguousarray(np.asarray(feature, dtype=np.float32))
    weight = np.ascontiguousarray(np.asarray(weight, dtype=np.float32))
    # fpk[core][par*64+e, bb*F+i] = feature[core*BLOC + 2*bb + par, i, e]
    ft = feature.reshape(NCORES, NPAIR, 2, F, E)  # [core, bb, par, i, e]
    fpk = (
        np.ascontiguousarray(ft.transpose(0, 2, 4, 1, 3))
        .reshape(NCORES * 128, NPAIR * F)
        .astype(np.float16)
    )
    # block-diagonal: wpk_one[e+64h, i*128 + h*64 + d] = w[i,e,d]
    wbd = np.zeros((2, E, F, 2, E), np.float16)  # [h, e, i, h', d]
    wt = weight.transpose(1, 0, 2).astype(np.float16)  # [e, i, d]
    wbd[0, :, :, 0, :] = wt
    wbd[1, :, :, 1, :] = wt
    wpk_one = wbd.reshape(128, F * 2 * E)
    wpk = np.tile(wpk_one, (NCORES, 1))
    return {"fpk": fpk, "wpk": wpk}


def unpack_output(out_dev):
    """out_dev: (8*128, 2, NPAIR, 2, F) device layout -> (B, F, F)."""
    o = out_dev.reshape(NCORES, 128, 2, NPAIR, 2, F)
    oA = o[:, :, 0].transpose(0, 2, 3, 1, 4)          # [c, bb, par, i(128), j]
    oB = o[:, :C2, 1].transpose(0, 2, 3, 1, 4)        # [c, bb, par, q(72), j]
    out = np.concatenate(
        [oA.reshape(NCORES, BLOC, C1, F), oB.reshape(NCORES, BLOC, C2, F)],
        axis=2,
    )
    return np.ascontiguousarray(out).reshape(B, F, F).astype(np.float32)


def kernel(feature, weight):
    r = _get_runner()
    ins = pack_inputs(feature, weight)
    outs = r.run(ins)
    return unpack_output(outs["out"])


if __name__ == "__main__":
    rng = np.random.default_rng(0)
    feature = rng.standard_normal((B, F, E), dtype=np.float32)
    weight = (0.01 * rng.standard_normal((F, E, E))).astype(np.float32)
    got = kernel(feature, weight)
    bil = np.einsum(
        "bie,ied->bid", feature.astype(np.float64), weight.astype(np.float64)
    )
    ref = np.einsum("bid,bjd->bij", bil, feature.astype(np.float64))
    err = np.abs(got - ref)
    denom = np.abs(ref).max()
    print("max abs err:", err.max(), "rel(scale):", err.max() / denom)
    l2 = np.linalg.norm((got - ref).ravel()) / np.linalg.norm(ref.ravel())
    print("L2 rel:", l2)
